# revision 1
# baseline (speedup 1.0000x reference)
"""Trainium2 Bass kernel for nn_Kernel_503460026817608471_12541304504791.

Full-input contract: kernel(x=(64,512,512) f32, p1_w=(512,1) f32,
p7_w=(1,512,512) f32) -> (64,512,512) f32.

Strategy: pure data parallel over the batch dim (8 items per NeuronCore).
Per batch item n (C = H = 512, P = 128, 4 partition chunks per matrix):

  t1  = p1.x          (PE matvec, [1,512])
  t2  = roll(x,2,c)   (C-layout: SBUF->SBUF DMAs; H-layout: free-dim shift)
  t3  = t1 + t2       (H: per-partition bias add; C: ones-matmul broadcast)
  t4  = x*t3, t5 = relu(x), t7 = p7*t5, t11 = min(t4, x)
  9 512^3 matmuls with layouts chosen so only x, t10, t14 ever need
  explicit PE transposes; 4 softmaxes fused into PSUM evacuation
  (DVE reduce_max -> ACT exp with accum_out row-sum -> DVE reciprocal+scale).

Matmul inputs are float32r (PE runs 1 cycle/row vs 4 for fp32); walrus
requires every f32r matmul operand to be produced by an instruction that
rounds to f32r, so all producer ops write f32r-dtype tiles (bit-compatible
with fp32 for non-matmul readers via bitcast).

All scale factors (1/sqrt(c), 1/sqrt(h)) are folded into evacuation scales
or softmax exp scales.
"""

import math
import os
import sys

for _p in ("/opt/trn_rl_repo",):
    if _p not in sys.path and os.path.isdir(_p):
        sys.path.insert(0, _p)

import numpy as np

import concourse.bass as bass
import concourse.tile as tile
from concourse import bacc
from concourse import mybir
from concourse import bass_utils
from concourse.masks import make_identity

N, C, H = 64, 512, 512
SHIFT = 2
NCORES = 8
NB = N // NCORES          # batch items per core
P = 128
KC = C // P               # 4 chunks along c
KH = H // P               # 4 chunks along h

F32 = mybir.dt.float32
F32R = mybir.dt.float32r
BF16 = mybir.dt.bfloat16
AF = mybir.ActivationFunctionType
ALU = mybir.AluOpType
AX = mybir.AxisListType

# matmul input dtype knob: f32r (1 cyc/row, ~fp32-ish storage) | bf16 | f32
_MMDT = os.environ.get("BASS_MM_DT", "f32r")
MDT = {"f32r": F32R, "bf16": BF16, "f32": F32}[_MMDT]
# whether a DMA write into an MDT tile is accepted as a rounding producer
DMA_MDT_OK = (MDT in (F32R, F32)) and \
    os.environ.get("BASS_DMA_MDT_OK", "1") == "1"

INV_SC = 1.0 / math.sqrt(C)
INV_SH = 1.0 / math.sqrt(H)


def _f(ap):
    """Read view of an MDT tile for elementwise ops."""
    return ap.bitcast(F32) if MDT is F32R else ap


def _t(ap):
    """Transpose PSUM output view (must match lhsT dtype)."""
    return ap.bitcast(MDT) if MDT is not F32 else ap


def build_program():
    nc = bacc.Bacc("TRN2", target_bir_lowering=False, debug=False,
                   num_devices=NCORES)
    x = nc.dram_tensor("x", [NB, C, H], F32, kind="ExternalInput").ap()
    p1 = nc.dram_tensor("p1", [C, 1], F32, kind="ExternalInput").ap()
    p7 = nc.dram_tensor("p7", [C, H], F32, kind="ExternalInput").ap()
    out = nc.dram_tensor("out", [NB, C, H], F32, kind="ExternalOutput").ap()

    with tile.TileContext(nc) as tc:
        _build_body(nc, tc, x, p1, p7, out)
    nc.compile()
    return nc


def _build_body(nc, tc, x, p1, p7, out):
    from contextlib import ExitStack
    ctx = ExitStack()
    constp = ctx.enter_context(tc.tile_pool(name="const", bufs=1))
    big = ctx.enter_context(tc.tile_pool(name="big", bufs=1))
    xpool = ctx.enter_context(tc.tile_pool(name="xc", bufs=2))
    small = ctx.enter_context(tc.tile_pool(name="small", bufs=4))
    psmm = ctx.enter_context(
        tc.tile_pool(name="psmm", bufs=4, space="PSUM"))
    pstt = ctx.enter_context(
        tc.tile_pool(name="pstt", bufs=2, space="PSUM"))
    psmisc = ctx.enter_context(
        tc.tile_pool(name="psmisc", bufs=1, space="PSUM"))

    identf = constp.tile([P, P], F32)
    make_identity(nc, identf[:])
    ident = constp.tile([P, P], MDT)
    nc.scalar.activation(ident[:], identf[:], AF.Identity)
    onesf = constp.tile([1, P], F32)
    nc.vector.memset(onesf[:], 1.0)
    ones1 = constp.tile([1, P], MDT)
    nc.scalar.activation(ones1[:], onesf[:], AF.Identity)

    p1_f = constp.tile([P, KC], F32)
    nc.sync.dma_start(p1_f[:], p1.rearrange("(k p) o -> p (k o)", p=P))
    p1_t = constp.tile([P, KC], MDT)
    nc.scalar.activation(p1_t[:], p1_f[:], AF.Identity)
    p7c = constp.tile([P, KC, H], F32)
    for k in range(KC):
        nc.sync.dma_start(p7c[:, k, :], p7[k * P:(k + 1) * P, :])

    # p7 transposed once per core (only feeds DVE muls; plain f32)
    p7h = constp.tile([P, KH, C], F32)
    for j in range(KH):
        ps = pstt.tile([P, C], F32, tag="pstt")
        for i in range(KC):
            nc.tensor.transpose(ps[:, i * P:(i + 1) * P],
                                p7c[:, i, j * P:(j + 1) * P], identf[:])
        nc.scalar.activation(p7h[:, j, :], ps[:], AF.Identity)

    def mm512(lhsT, rhs, consume):
        """out[m,n] = sum_k lhsT[k,m] * rhs[k,n], 512^3, via 4x4 PE matmuls.
        lhsT/rhs are [P, 4, 512] chunk-major MDT tiles; consume(m, psum)."""
        for m in range(4):
            ps = psmm.tile([P, 512], F32, tag="psmm")
            for k in range(4):
                nc.tensor.matmul(ps[:],
                                 lhsT[:, k, m * P:(m + 1) * P],
                                 rhs[:, k, :],
                                 start=(k == 0), stop=(k == 3))
            consume(m, ps)

    def evac(dst, scale):
        def f(m, ps):
            nc.scalar.activation(dst[:, m, :], ps[:], AF.Identity,
                                 scale=scale)
        return f

    def softmax(dst, scale):
        """Row softmax of (scale * psum) evacuated into dst chunk m."""
        def f(m, ps):
            rmax = small.tile([P, 1], F32, tag="rmax")
            nc.vector.reduce_max(rmax[:], ps[:], axis=AX.X)
            nbias = small.tile([P, 1], F32, tag="nbias")
            nc.scalar.activation(nbias[:], rmax[:], AF.Identity,
                                 scale=-scale)
            esum = small.tile([P, 1], F32, tag="esum")
            nc.scalar.activation(dst[:, m, :], ps[:], AF.Exp,
                                 bias=nbias[:], scale=scale,
                                 accum_out=esum[:])
            rr = small.tile([P, 1], F32, tag="rr")
            nc.vector.reciprocal(rr[:], esum[:])
            nc.vector.tensor_scalar_mul(dst[:, m, :], _f(dst[:, m, :]),
                                        rr[:])
        return f

    def transpose512(src, dst):
        for j in range(4):
            ps = pstt.tile([P, 512], F32, tag="pstt")
            for i in range(4):
                nc.tensor.transpose(ps[:, i * P:(i + 1) * P],
                                    _f(src[:, i, j * P:(j + 1) * P]),
                                    identf[:])
            nc.vector.tensor_copy(dst[:, j, :], ps[:])

    for n in range(NB):
        if DMA_MDT_OK:
            xc = xpool.tile([P, KC, H], MDT, tag="xc")
            for k in range(KC):
                nc.sync.dma_start(xc[:, k, :],
                                  x[n][k * P:(k + 1) * P, :].bitcast(MDT))
        else:
            xcf = xpool.tile([P, KC, H], F32, tag="xcf")
            nc.sync.dma_start(xcf[:],
                              x[n].rearrange("(k p) h -> p k h", p=P))
            xc = xpool.tile([P, KC, H], MDT, tag="xc")
            for k in range(KC):
                nc.scalar.activation(xc[:, k, :], xcf[:, k, :], AF.Identity)

        # t1 = p1 . x  -> [1, H] psum, evac to SBUF
        ps_t1 = psmisc.tile([1, H], F32, tag="psmisc")
        for k in range(KC):
            nc.tensor.matmul(ps_t1[:], p1_t[:, k:k + 1], xc[:, k, :],
                             start=(k == 0), stop=(k == KC - 1))
        t1row = small.tile([1, H], MDT, tag="t1row")
        nc.scalar.activation(t1row[:], ps_t1[:], AF.Identity)

        # t1 as per-partition column [P, KH] (for H-layout bias adds)
        ps_t1h = psmisc.tile([P, KH], F32, tag="psmisc")
        for j in range(KH):
            nc.tensor.transpose(ps_t1h[:, j:j + 1],
                                _f(t1row[0:1, j * P:(j + 1) * P]),
                                identf[0:1, 0:1])
        t1h = small.tile([P, KH], F32, tag="t1h")
        nc.vector.tensor_copy(t1h[:], ps_t1h[:])

        # x transposed: xh[h, c]
        xh = big.tile([P, KH, C], MDT, tag="G")
        for j in range(KH):
            ps = pstt.tile([P, C], F32, tag="pstt")
            for i in range(KC):
                nc.tensor.transpose(ps[:, i * P:(i + 1) * P],
                                    _f(xc[:, i, j * P:(j + 1) * P]),
                                    identf[:])
            nc.scalar.activation(xh[:, j, :], ps[:], AF.Identity)

        # H-layout elementwise
        t5h = big.tile([P, KH, C], MDT, tag="E")
        t3h = big.tile([P, KH, C], MDT, tag="Cg")
        t4h = big.tile([P, KH, C], MDT, tag="D")
        t7h = big.tile([P, KH, C], MDT, tag="F")
        for j in range(KH):
            nc.scalar.activation(t5h[:, j, :], _f(xh[:, j, :]), AF.Relu)
            # t3h = roll(x,2,axis=c) + t1  (roll => free-dim shift in H)
            nc.vector.tensor_scalar_add(t3h[:, j, SHIFT:C],
                                        _f(xh[:, j, 0:C - SHIFT]),
                                        t1h[:, j:j + 1])
            nc.vector.tensor_scalar_add(t3h[:, j, 0:SHIFT],
                                        _f(xh[:, j, C - SHIFT:C]),
                                        t1h[:, j:j + 1])
            nc.vector.tensor_mul(t4h[:, j, :], _f(xh[:, j, :]),
                                 _f(t3h[:, j, :]))
            nc.vector.tensor_mul(t7h[:, j, :], p7h[:, j, :],
                                 _f(t5h[:, j, :]))

        # C-layout: t2 = roll(x, 2, axis=c) via partition-shifting DMAs
        if DMA_MDT_OK:
            t2c = big.tile([P, KC, H], MDT, tag="t2c")
            for k in range(KC):
                nc.sync.dma_start(t2c[SHIFT:P, k, :],
                                  xc[0:P - SHIFT, k, :])
                nc.sync.dma_start(t2c[0:SHIFT, k, :],
                                  xc[P - SHIFT:P, (k - 1) % KC, :])
        else:
            t2cf = big.tile([P, KC, H], F32, tag="t2cf")
            for k in range(KC):
                nc.sync.dma_start(t2cf[SHIFT:P, k, :],
                                  _f(xc[0:P - SHIFT, k, :]))
                nc.sync.dma_start(t2cf[0:SHIFT, k, :],
                                  _f(xc[P - SHIFT:P, (k - 1) % KC, :]))
            t2c = big.tile([P, KC, H], MDT, tag="t2c")
            for k in range(KC):
                nc.scalar.activation(t2c[:, k, :], t2cf[:, k, :],
                                     AF.Identity)

        # broadcast t1 row across partitions via ones-matmul
        ps_bc = psmisc.tile([P, H], F32, tag="psbc")
        nc.tensor.matmul(ps_bc[:], ones1[:], t1row[:],
                         start=True, stop=True)

        t5c = big.tile([P, KC, H], MDT, tag="Hg")
        t7c = big.tile([P, KC, H], MDT, tag="t7c")
        t3c = big.tile([P, KC, H], F32, tag="A")
        t4c = big.tile([P, KC, H], F32, tag="B")
        t11c = big.tile([P, KC, H], MDT, tag="t11c")
        for k in range(KC):
            nc.scalar.activation(t5c[:, k, :], _f(xc[:, k, :]), AF.Relu)
            nc.vector.tensor_mul(t7c[:, k, :], p7c[:, k, :],
                                 _f(t5c[:, k, :]))
            nc.vector.tensor_add(t3c[:, k, :], _f(t2c[:, k, :]), ps_bc[:])
            nc.vector.tensor_mul(t4c[:, k, :], _f(xc[:, k, :]),
                                 t3c[:, k, :])
            nc.vector.tensor_tensor(t11c[:, k, :], t4c[:, k, :],
                                    _f(xc[:, k, :]), op=ALU.min)

        # MM1: t8 = softmax(t5^T x / sqrt(c))           [h, g]
        t8 = big.tile([P, KH, H], MDT, tag="t8")
        mm512(t5c, xc, softmax(t8, INV_SC))
        # MM2: t9_raw = t5^T_h t4_h   (true t9 = raw * inv_sh)   [c, d]
        t9 = big.tile([P, KC, C], MDT, tag="t9")
        mm512(t5h, t4h, evac(t9, 1.0))
        # MM3: t10 = softmax(t7 t3^T / sqrt(h))          [c, d]
        t10 = big.tile([P, KC, C], MDT, tag="t10")
        mm512(t7h, t3h, softmax(t10, INV_SH))
        t10T = big.tile([P, KC, C], MDT, tag="B2")
        transpose512(t10, t10T)
        # MM4: t12 = t9^T t7 / (sqrt(c) sqrt(h))         [i, h]
        t12 = big.tile([P, KC, H], MDT, tag="F")
        mm512(t9, t7c, evac(t12, INV_SC * INV_SH))
        # MM5: t13T_raw = t2^T t11   (true t13 = raw^T * inv_sc) [g, h]
        t13T = big.tile([P, KH, H], MDT, tag="G")
        mm512(t2c, t11c, evac(t13T, 1.0))
        # MM6: t15 = softmax(t13 t8 / (sqrt(c) sqrt(h)))  [i, k]
        t15 = big.tile([P, KH, H], MDT, tag="Cg")
        mm512(t13T, t8, softmax(t15, INV_SC * INV_SH))
        # MM7: t14 = softmax(t9^T t10^T / (sqrt(c) sqrt(h))) [i, j]
        t14 = big.tile([P, KC, C], MDT, tag="E")
        mm512(t9, t10T, softmax(t14, INV_SC * INV_SH))
        t14T = big.tile([P, KC, C], MDT, tag="A2")
        transpose512(t14, t14T)
        # MM8: t16 = t12^T t14T / sqrt(c)                [h, j]
        t16 = big.tile([P, KH, C], MDT, tag="D")
        mm512(t12, t14T, evac(t16, INV_SC))
        # MM9: t17 = t16^T t15 / sqrt(h)                 [c, j]
        t17 = big.tile([P, KC, H], F32, tag="Hg")
        mm512(t16, t15, evac(t17, INV_SH))

        for k in range(KC):
            nc.sync.dma_start(out[n][k * P:(k + 1) * P, :], t17[:, k, :])

    ctx.close()


_NC_CACHE = {}


def _get_program():
    if "nc" not in _NC_CACHE:
        _NC_CACHE["nc"] = build_program()
    return _NC_CACHE["nc"]


def kernel(x, p1_w, p7_w):
    x = np.ascontiguousarray(x, dtype=np.float32)
    p1 = np.ascontiguousarray(p1_w, dtype=np.float32)
    p7 = np.ascontiguousarray(np.asarray(p7_w).reshape(C, H),
                              dtype=np.float32)
    nc = _get_program()
    in_maps = [
        {"x": np.ascontiguousarray(x[i * NB:(i + 1) * NB]),
         "p1": p1, "p7": p7}
        for i in range(NCORES)
    ]
    res = bass_utils.run_bass_kernel_spmd(nc, in_maps,
                                          core_ids=list(range(NCORES)))
    outs = [np.asarray(res.results[i]["out"]) for i in range(NCORES)]
    return np.concatenate(outs, axis=0)


if __name__ == "__main__":
    nc = build_program()
    print("built ok")



# revision 48
# speedup vs baseline: 1.3051x; 1.3051x over previous
"""Trainium2 Bass kernel for nn_Kernel_503460026817608471_12541304504791.

Full-input contract: kernel(x=(64,512,512) f32, p1_w=(512,1) f32,
p7_w=(1,512,512) f32) -> (64,512,512) f32.

Pure data parallel over batch (8 items per NeuronCore). Per item
(C = H = 512, P = 128, 4 partition chunks per matrix):

  t1  = p1.x          (PE matvec)
  t2  = roll(x,2,c)   (C-layout: 3 partition-shifting SBUF DMAs)
  t3  = t1 + t2; t4 = x*t3; t5 = relu(x); t7 = p7*t5; t11 = min(t4,x)
  9 512^3 matmuls (f32r / bf16 operands, 1 PE cycle/row), 4 softmaxes.

Softmaxes skip max-subtraction (logits are bounded, max ~26) and their
row normalizers are folded into downstream per-partition evac scales:
  r8  -> t13T evac   (t8 used as MM6 rhs; t13T rows share the index)
  r15 -> t16 evac    (t15 used as MM9 rhs)
  r14 -> t17 evac    (t14T free dim becomes MM9 psum partition)
  r10 -> explicit in-place mul on GpSimd (transpose blocks folding)

Engine balance: PE ~35us/item (bottleneck), ACT exps+evacs, DVE
elementwise+evacs, GpSimd (Pool) p7 muls + t10 norm, Sync 6 DMAs.
"""

import math
import os
import sys

for _p in ("/opt/trn_rl_repo",):
    if _p not in sys.path and os.path.isdir(_p):
        sys.path.insert(0, _p)

import numpy as np

import concourse.bass as bass
import concourse.tile as tile
from concourse import bacc
from concourse import mybir
from concourse import bass_utils
from concourse.masks import make_identity

N, C, H = 64, 512, 512
SHIFT = 2
NCORES = 8
NB = N // NCORES          # batch items per core
P = 128
KC = C // P               # 4 chunks along c
KH = H // P               # 4 chunks along h

F32 = mybir.dt.float32
F32R = mybir.dt.float32r
BF16 = mybir.dt.bfloat16
AF = mybir.ActivationFunctionType
ALU = mybir.AluOpType
AX = mybir.AxisListType

MDT = F32R
SC = math.sqrt(C)
SH = math.sqrt(H)
INV_SC = 1.0 / SC
INV_SH = 1.0 / SH

# Config knobs. Defaults chosen from HW bisection: XBAR DMA-transposes
# corrupt data when concurrent with other DMA traffic in this kernel
# (known TRN2 erratum: Tile normally serializes them), and without
# them bf16 matmul operands buy nothing (PE is 1 cycle/row for f32r
# and bf16 alike, while DVE *reads* of bf16 are ~4x slower).
NO_DMAT = os.environ.get("K_NO_DMAT", "1") == "1"   # PE transposes
NO_POOL = os.environ.get("K_NO_POOL", "0") == "1"   # DVE muls instead
NO_BF16 = os.environ.get("K_NO_BF16", "1") == "1"   # f32r matmuls only
MMDT = F32R if NO_BF16 else BF16


def _f(ap):
    """f32 read view of an f32r tile for elementwise ops."""
    return ap.bitcast(F32)


def _t(ap):
    """Transpose PSUM output view (must match transpose input dtype)."""
    return ap.bitcast(MDT)


def build_program():
    nc = bacc.Bacc("TRN2", target_bir_lowering=False, debug=False,
                   num_devices=NCORES)
    x = nc.dram_tensor("x", [NB, C, H], F32, kind="ExternalInput").ap()
    p1 = nc.dram_tensor("p1", [C, 1], F32, kind="ExternalInput").ap()
    p7 = nc.dram_tensor("p7", [C, H], F32, kind="ExternalInput").ap()
    out = nc.dram_tensor("out", [NB, C, H], F32, kind="ExternalOutput").ap()

    with tile.TileContext(nc) as tc:
        _build_body(nc, tc, x, p1, p7, out)
    nc.compile()
    return nc


def _build_body(nc, tc, x, p1, p7, out):
    from contextlib import ExitStack
    ctx = ExitStack()
    constp = ctx.enter_context(tc.tile_pool(name="const", bufs=1))
    big = ctx.enter_context(tc.tile_pool(name="big", bufs=1))
    xpool = ctx.enter_context(tc.tile_pool(name="xc", bufs=2))
    small = ctx.enter_context(tc.tile_pool(name="small", bufs=2))
    psmm = ctx.enter_context(
        tc.tile_pool(name="psmm", bufs=5, space="PSUM"))
    pstt = ctx.enter_context(
        tc.tile_pool(name="pstt", bufs=2, space="PSUM"))
    psmisc = ctx.enter_context(
        tc.tile_pool(name="psmisc", bufs=1, space="PSUM"))

    identf = constp.tile([P, P], F32)
    make_identity(nc, identf[:])
    ident = constp.tile([P, P], MDT)
    nc.scalar.activation(ident[:], identf[:], AF.Identity)

    p1_f = constp.tile([P, KC], F32)
    nc.sync.dma_start(p1_f[:], p1.rearrange("(k p) o -> p (k o)", p=P))
    p1_t = constp.tile([P, KC], MDT)
    nc.scalar.activation(p1_t[:], p1_f[:], AF.Identity)
    p7c = constp.tile([P, KC, H], F32)
    nc.sync.dma_start(p7c[:], p7.rearrange("(k p) h -> p k h", p=P))

    # p7 transposed once per core (feeds Pool muls; plain f32)
    p7h = constp.tile([P, KH, C], F32)
    for j in range(KH):
        ps = pstt.tile([P, C], F32, tag="pstt")
        for i in range(KC):
            nc.tensor.transpose(ps[:, i * P:(i + 1) * P],
                                p7c[:, i, j * P:(j + 1) * P], identf[:])
        nc.scalar.activation(p7h[:, j, :], ps[:], AF.Identity)

    def mm512(lhsT, rhs, consume):
        """out[m,n] = sum_k lhsT[k,m] * rhs[k,n], 512^3, via 4x4 PE matmuls.
        lhsT/rhs are [P, 4, 512] chunk-major tiles; consume(m, psum)."""
        for m in range(4):
            ps = psmm.tile([P, 512], F32, tag="psmm")
            for k in range(4):
                nc.tensor.matmul(ps[:],
                                 lhsT[:, k, m * P:(m + 1) * P],
                                 rhs[:, k, :],
                                 start=(k == 0), stop=(k == 3))
            consume(m, ps)

    def evac_act(dst, scale):
        def f(m, ps):
            nc.scalar.activation(dst[:, m, :], ps[:], AF.Identity,
                                 scale=scale)
        return f

    def evac_act_ap(dst, rr):
        """ACT evac scaled by per-partition column rr[:, m]."""
        def f(m, ps):
            nc.scalar.activation(dst[:, m, :], ps[:], AF.Copy,
                                 scale=rr[:, m:m + 1])
        return f

    def evac_dve(dst, scale):
        def f(m, ps):
            if scale == 1.0:
                nc.vector.tensor_copy(dst[:, m, :], ps[:])
            else:
                nc.vector.tensor_scalar_mul(dst[:, m, :], ps[:], scale)
        return f

    def expevac(dst, scale, esum):
        """dst[:,m,:] = exp(scale*psum); row sums accumulated in esum."""
        def f(m, ps):
            nc.scalar.activation(dst[:, m, :], ps[:], AF.Exp,
                                 scale=scale, accum_out=esum[:, m:m + 1])
        return f

    def transpose512(src, dst, evac):
        """dst = src^T (512x512) via 16 PE transposes; evac(j, psum)."""
        for j in range(4):
            ps = pstt.tile([P, 512], F32, tag="pstt")
            for i in range(4):
                nc.tensor.transpose(_t(ps[:, i * P:(i + 1) * P]),
                                    src[:, i, j * P:(j + 1) * P],
                                    ident[:])
            evac(j, ps)

    def dma_transpose512(src, dst):
        """dst = src^T (512x512, 2-byte dtype) via 4 XBAR DMA transposes."""
        for k in range(4):
            nc.sync.dma_start_transpose(
                dst[:, :, k * P:(k + 1) * P], src[:, k, :])

    xcs = {}

    def load_xc(n):
        t = xpool.tile([P, KC, H], MDT, tag="xc", name="xc")
        for k in range(KC):
            nc.sync.dma_start(t[:, k, :],
                              x[n][k * P:(k + 1) * P, :].bitcast(MDT))
        xcs[n] = t

    heads = {}

    def do_head(n):
        """Emit item n's preamble: t1 chain + x transpose (PE-heavy).

        Called early for item 0, then from inside item n-1's MM7->MM8
        latency gap so the PE never idles at item boundaries.
        """
        xc = xcs[n]
        # t1 = p1 . x  -> [1, H] psum
        ps_t1 = psmisc.tile([1, H], F32, tag="psmisc", name="ps_t1")
        for k in range(KC):
            nc.tensor.matmul(ps_t1[:], p1_t[:, k:k + 1], xc[:, k, :],
                             start=(k == 0), stop=(k == KC - 1))
        t1row = small.tile([1, H], MDT, tag="t1row", name="t1row")
        nc.scalar.activation(t1row[:], ps_t1[:], AF.Identity)

        # t1 as per-partition column [P, KH] (f32 transpose: f32r is
        # invalid ISA for tiny single-column transposes)
        ps_t1h = psmisc.tile([P, KH], F32, tag="psmisc", name="ps_t1h")
        for j in range(KH):
            nc.tensor.transpose(ps_t1h[:, j:j + 1],
                                _f(t1row[0:1, j * P:(j + 1) * P]),
                                identf[0:1, 0:1])
        t1h = small.tile([P, KH], F32, tag="t1h", name="t1h")
        nc.vector.tensor_copy(t1h[:], ps_t1h[:])

        # x transposed on PE: xh[h, c]; t5h = relu from the same psum
        xh = big.tile([P, KH, C], MDT, tag="xh", name="xh")
        t5h = big.tile([P, KH, C], MDT, tag="t5h", name="t5h")
        for j in range(KH):
            ps = pstt.tile([P, C], F32, tag="pstt", name="ps_xh")
            for i in range(KC):
                nc.tensor.transpose(_t(ps[:, i * P:(i + 1) * P]),
                                    xc[:, i, j * P:(j + 1) * P],
                                    ident[:])
            nc.vector.tensor_copy(xh[:, j, :], ps[:])
            nc.vector.tensor_scalar_max(t5h[:, j, :], ps[:], 0.0)
        heads[n] = (t1h, xh, t5h)

    mm1s = {}

    def do_mm1(n):
        """Emit t5c/t7c (Pool) + MM1 for item n; called from inside item
        n-1's MM7->MM8 latency gap to keep the PE fed."""
        xc = xcs[n]
        t5c = big.tile([P, KC, H], MDT, tag="t5c", name="t5c")
        t7c = big.tile([P, KC, H], MMDT, tag="t7c", name="t7c")
        for k in range(KC):
            nc.scalar.activation(t5c[:, k, :], _f(xc[:, k, :]), AF.Relu)
            eng_mul = nc.vector if NO_POOL else nc.gpsimd
            eng_mul.tensor_mul(t7c[:, k, :], p7c[:, k, :],
                               _f(t5c[:, k, :]))
        esum8 = small.tile([P, KH], F32, tag="esum8", name="esum8")
        # MM1: t8_un = exp(t5^T x / sqrt(c))           [h, g]
        t8 = big.tile([P, KH, H], MMDT, tag="t8", name="t8")
        mm512(t5c, xc, expevac(t8, INV_SC, esum8))
        rr8 = small.tile([P, KH], F32, tag="rr8", name="rr8")
        nc.vector.reciprocal(rr8[:], esum8[:])
        mm1s[n] = (t7c, t8, rr8)

    load_xc(0)
    do_head(0)
    do_mm1(0)
    for n in range(NB):
        xc = xcs.pop(n)
        t1h, xh, t5h = heads.pop(n)
        t7c, t8, rr8 = mm1s.pop(n)

        # prefetch next item's x while this item computes
        if n + 1 < NB:
            load_xc(n + 1)

        # H-layout elementwise (f32r: bf16 reads are slow on DVE)
        t3h = big.tile([P, KH, C], MDT, tag="t3h")
        t4h = big.tile([P, KH, C], MDT, tag="t4h")
        t7h = big.tile([P, KH, C], MDT, tag="t7h")
        t11h = big.tile([P, KH, C], MDT, tag="t11h")
        for j in range(KH):
            # t3h = roll(x,2,axis=c) + t1  (roll => free-dim shift in H)
            nc.vector.tensor_scalar_add(t3h[:, j, SHIFT:C],
                                        _f(xh[:, j, 0:C - SHIFT]),
                                        t1h[:, j:j + 1])
            nc.vector.tensor_scalar_add(t3h[:, j, 0:SHIFT],
                                        _f(xh[:, j, C - SHIFT:C]),
                                        t1h[:, j:j + 1])
            nc.vector.tensor_mul(t4h[:, j, :], _f(xh[:, j, :]),
                                 _f(t3h[:, j, :]))
            eng_mul = nc.vector if NO_POOL else nc.gpsimd
            eng_mul.tensor_mul(t7h[:, j, :], p7h[:, j, :],
                               _f(t5h[:, j, :]))
            nc.vector.tensor_tensor(t11h[:, j, :], _f(t4h[:, j, :]),
                                    _f(xh[:, j, :]), op=ALU.min)

        # t2 = roll(x, 2, axis=c): loaded straight from DRAM with row
        # shift (NO SBUF->SBUF DMA: those deadlock against XBAR
        # DMA-transposes on HW)
        t2c = big.tile([P, KC, H], MDT, tag="t2c")
        nc.sync.dma_start(
            t2c[SHIFT:P, 0, :],
            x[n][0:P - SHIFT, :].bitcast(MDT))
        nc.sync.dma_start(
            t2c[:, 1:KC, :],
            x[n][P - SHIFT:C - SHIFT, :].rearrange(
                "(k p) h -> p k h", p=P).bitcast(MDT))
        nc.sync.dma_start(
            t2c[0:SHIFT, 0, :],
            x[n][C - SHIFT:C, :].bitcast(MDT))

        # t11c = t11h^T on PE
        t11c = big.tile([P, KC, H], MDT, tag="t11c")
        transpose512(t11h, t11c, evac_dve(t11c, 1.0))

        # softmax row-sum tiles
        esum10 = small.tile([P, KC], F32, tag="esum10")
        esum15 = small.tile([P, KH], F32, tag="esum15")
        esum14 = small.tile([P, KC], F32, tag="esum14")

        # MM2: t9_raw = t5^T_h t4_h  (true t9 = raw * inv_sh)  [c, d]
        t9 = big.tile([P, KC, C], MMDT, tag="t9")
        mm512(t5h, t4h, evac_dve(t9, 1.0))

        # MM3: t10_un = exp(t7 t3^T / sqrt(h)); norm on Pool    [c, d]
        t10 = big.tile([P, KC, C], MMDT, tag="t10")
        mm512(t7h, t3h, expevac(t10, INV_SH, esum10))
        rr10 = small.tile([P, KC], F32, tag="rr10")
        nc.vector.reciprocal(rr10[:], esum10[:])
        for m in range(KC):
            nc.scalar.mul(t10[:, m, :], t10[:, m, :], rr10[:, m:m + 1])

        # MM5: t13T_un = t2^T t11 * r8-rows              [g, h]
        t13T = big.tile([P, KH, H], MMDT, tag="t13T")
        mm512(t2c, t11c, evac_act_ap(t13T, rr8))

        # t10T = t10^T (normalized)
        t10T = big.tile([P, KC, C], MMDT, tag="t10T")
        if NO_DMAT:
            transpose512(t10, t10T, evac_dve(t10T, 1.0))
        else:
            dma_transpose512(t10, t10T)

        # MM4: t12 = t9^T t7 / (sqrt(c) sqrt(h))         [i, h]
        t12 = big.tile([P, KC, H], MMDT, tag="t12")
        mm512(t9, t7c, evac_dve(t12, INV_SC * INV_SH))

        # MM6: t15_un = exp(t13 t8 / (sqrt(c) sqrt(h)))  [i, k]
        t15 = big.tile([P, KH, H], MDT, tag="t15")
        mm512(t13T, t8, expevac(t15, INV_SC * INV_SH, esum15))
        # rr15c = 1/(esum15*sqrt(c)) folds MM8's inv_sc with r15
        esum15s = small.tile([P, KH], F32, tag="esum15s")
        nc.vector.tensor_scalar_mul(esum15s[:], esum15[:], SC)
        rr15c = small.tile([P, KH], F32, tag="rr15c")
        nc.vector.reciprocal(rr15c[:], esum15s[:])

        # MM7: t14_un = exp(t9^T t10^T / (sqrt(c) sqrt(h))) [i, j]
        t14 = big.tile([P, KC, C], MMDT, tag="t14")
        mm512(t9, t10T, expevac(t14, INV_SC * INV_SH, esum14))
        esum14s = small.tile([P, KC], F32, tag="esum14s")
        nc.vector.tensor_scalar_mul(esum14s[:], esum14[:], SH)
        rr14c = small.tile([P, KC], F32, tag="rr14c")
        nc.vector.reciprocal(rr14c[:], esum14s[:])

        # fill the MM7->t14T latency gap with the next item's preamble
        # and its MM1 (PE work that hides the t14T DMA latency)
        if n + 1 < NB:
            do_head(n + 1)
            do_mm1(n + 1)

        # t14T
        t14T = big.tile([P, KC, C], MMDT, tag="t14T")
        if NO_DMAT:
            transpose512(t14, t14T, evac_dve(t14T, 1.0))
        else:
            dma_transpose512(t14, t14T)

        # MM8: t16 = t12^T t14T * (inv_sc * r15-rows)    [h, j]
        t16 = big.tile([P, KH, C], MDT, tag="t16")
        mm512(t12, t14T, evac_act_ap(t16, rr15c))

        # MM9: t17 = t16^T t15 * (inv_sh * r14-rows); per-chunk DMA out
        t17 = big.tile([P, KC, H], F32, tag="t17")

        def evac_out(m, ps):
            nc.scalar.activation(t17[:, m, :], ps[:], AF.Copy,
                                 scale=rr14c[:, m:m + 1])
            nc.sync.dma_start(out[n][m * P:(m + 1) * P, :], t17[:, m, :])

        mm512(t16, t15, evac_out)

    ctx.close()


_NC_CACHE = {}


def _get_program():
    if "nc" not in _NC_CACHE:
        _NC_CACHE["nc"] = build_program()
    return _NC_CACHE["nc"]


def kernel(x, p1_w, p7_w):
    x = np.ascontiguousarray(x, dtype=np.float32)
    p1 = np.ascontiguousarray(p1_w, dtype=np.float32)
    p7 = np.ascontiguousarray(np.asarray(p7_w).reshape(C, H),
                              dtype=np.float32)
    nc = _get_program()
    in_maps = [
        {"x": np.ascontiguousarray(x[i * NB:(i + 1) * NB]),
         "p1": p1, "p7": p7}
        for i in range(NCORES)
    ]
    res = bass_utils.run_bass_kernel_spmd(nc, in_maps,
                                          core_ids=list(range(NCORES)))
    outs = [np.asarray(res.results[i]["out"]) for i in range(NCORES)]
    return np.concatenate(outs, axis=0)


if __name__ == "__main__":
    nc = build_program()
    print("built ok")


# revision 50
# speedup vs baseline: 1.3489x; 1.0335x over previous
"""Trainium2 Bass kernel for nn_Kernel_503460026817608471_12541304504791.

Full-input contract: kernel(x=(64,512,512) f32, p1_w=(512,1) f32,
p7_w=(1,512,512) f32) -> (64,512,512) f32.

Pure data parallel over batch (8 items per NeuronCore). Per item
(C = H = 512, P = 128, 4 partition chunks per matrix):

  t1  = p1.x          (PE matvec)
  t2  = roll(x,2,c)   (C-layout: 3 partition-shifting SBUF DMAs)
  t3  = t1 + t2; t4 = x*t3; t5 = relu(x); t7 = p7*t5; t11 = min(t4,x)
  9 512^3 matmuls (f32r / bf16 operands, 1 PE cycle/row), 4 softmaxes.

Softmaxes skip max-subtraction (logits are bounded, max ~26) and their
row normalizers are folded into downstream per-partition evac scales:
  r8  -> t13T evac   (t8 used as MM6 rhs; t13T rows share the index)
  r15 -> t16 evac    (t15 used as MM9 rhs)
  r14 -> t17 evac    (t14T free dim becomes MM9 psum partition)
  r10 -> explicit in-place mul on GpSimd (transpose blocks folding)

Engine balance: PE ~35us/item (bottleneck), ACT exps+evacs, DVE
elementwise+evacs, GpSimd (Pool) p7 muls + t10 norm, Sync 6 DMAs.
"""

import math
import os
import sys

for _p in ("/opt/trn_rl_repo",):
    if _p not in sys.path and os.path.isdir(_p):
        sys.path.insert(0, _p)

import numpy as np

import concourse.bass as bass
import concourse.tile as tile
from concourse import bacc
from concourse import mybir
from concourse import bass_utils
from concourse.masks import make_identity

N, C, H = 64, 512, 512
SHIFT = 2
NCORES = 8
NB = N // NCORES          # batch items per core
P = 128
KC = C // P               # 4 chunks along c
KH = H // P               # 4 chunks along h

F32 = mybir.dt.float32
F32R = mybir.dt.float32r
BF16 = mybir.dt.bfloat16
AF = mybir.ActivationFunctionType
ALU = mybir.AluOpType
AX = mybir.AxisListType

MDT = F32R
SC = math.sqrt(C)
SH = math.sqrt(H)
INV_SC = 1.0 / SC
INV_SH = 1.0 / SH

# Config knobs. Defaults chosen from HW bisection: XBAR DMA-transposes
# corrupt data when concurrent with other DMA traffic in this kernel
# (known TRN2 erratum: Tile normally serializes them), and without
# them bf16 matmul operands buy nothing (PE is 1 cycle/row for f32r
# and bf16 alike, while DVE *reads* of bf16 are ~4x slower).
NO_DMAT = os.environ.get("K_NO_DMAT", "1") == "1"   # PE transposes
NO_POOL = os.environ.get("K_NO_POOL", "0") == "1"   # DVE muls instead
NO_BF16 = os.environ.get("K_NO_BF16", "1") == "1"   # f32r matmuls only
MMDT = F32R if NO_BF16 else BF16


def _f(ap):
    """f32 read view of an f32r tile for elementwise ops."""
    return ap.bitcast(F32)


def _t(ap):
    """Transpose PSUM output view (must match transpose input dtype)."""
    return ap.bitcast(MDT)


def build_program():
    nc = bacc.Bacc("TRN2", target_bir_lowering=False, debug=False,
                   num_devices=NCORES)
    x = nc.dram_tensor("x", [NB, C, H], F32, kind="ExternalInput").ap()
    p1 = nc.dram_tensor("p1", [C, 1], F32, kind="ExternalInput").ap()
    p7 = nc.dram_tensor("p7", [C, H], F32, kind="ExternalInput").ap()
    out = nc.dram_tensor("out", [NB, C, H], F32, kind="ExternalOutput").ap()

    with tile.TileContext(nc) as tc:
        _build_body(nc, tc, x, p1, p7, out)
    nc.compile()
    return nc


def _build_body(nc, tc, x, p1, p7, out):
    from contextlib import ExitStack
    ctx = ExitStack()
    constp = ctx.enter_context(tc.tile_pool(name="const", bufs=1))
    big = ctx.enter_context(tc.tile_pool(name="big", bufs=1))
    xpool = ctx.enter_context(tc.tile_pool(name="xc", bufs=2))
    small = ctx.enter_context(tc.tile_pool(name="small", bufs=2))
    psmm = ctx.enter_context(
        tc.tile_pool(name="psmm", bufs=5, space="PSUM"))
    pstt = ctx.enter_context(
        tc.tile_pool(name="pstt", bufs=2, space="PSUM"))
    psmisc = ctx.enter_context(
        tc.tile_pool(name="psmisc", bufs=1, space="PSUM"))

    identf = constp.tile([P, P], F32)
    make_identity(nc, identf[:])
    ident = constp.tile([P, P], MDT)
    nc.scalar.activation(ident[:], identf[:], AF.Identity)
    identb = constp.tile([P, P], BF16)
    nc.scalar.activation(identb[:], identf[:], AF.Identity)

    p1_f = constp.tile([P, KC], F32)
    nc.sync.dma_start(p1_f[:], p1.rearrange("(k p) o -> p (k o)", p=P))
    p1_t = constp.tile([P, KC], MDT)
    nc.scalar.activation(p1_t[:], p1_f[:], AF.Identity)
    p7c = constp.tile([P, KC, H], F32)
    p7h = constp.tile([P, KH, C], F32)

    def mm512(lhsT, rhs, consume):
        """out[m,n] = sum_k lhsT[k,m] * rhs[k,n], 512^3, via 4x4 PE matmuls.
        lhsT/rhs are [P, 4, 512] chunk-major tiles; consume(m, psum)."""
        for m in range(4):
            ps = psmm.tile([P, 512], F32, tag="psmm")
            for k in range(4):
                nc.tensor.matmul(ps[:],
                                 lhsT[:, k, m * P:(m + 1) * P],
                                 rhs[:, k, :],
                                 start=(k == 0), stop=(k == 3))
            consume(m, ps)

    def evac_act(dst, scale):
        def f(m, ps):
            nc.scalar.activation(dst[:, m, :], ps[:], AF.Identity,
                                 scale=scale)
        return f

    def evac_act_ap(dst, rr):
        """ACT evac scaled by per-partition column rr[:, m]."""
        def f(m, ps):
            nc.scalar.activation(dst[:, m, :], ps[:], AF.Copy,
                                 scale=rr[:, m:m + 1])
        return f

    def evac_dve(dst, scale):
        def f(m, ps):
            if scale == 1.0:
                nc.vector.tensor_copy(dst[:, m, :], ps[:])
            else:
                nc.vector.tensor_scalar_mul(dst[:, m, :], ps[:], scale)
        return f

    def expevac(dst, scale, esum):
        """dst[:,m,:] = exp(scale*psum); row sums accumulated in esum."""
        def f(m, ps):
            nc.scalar.activation(dst[:, m, :], ps[:], AF.Exp,
                                 scale=scale, accum_out=esum[:, m:m + 1])
        return f

    def transpose512(src, dst, evac):
        """dst = src^T (512x512) via 16 PE transposes; evac(j, psum)."""
        for j in range(4):
            ps = pstt.tile([P, 512], F32, tag="pstt")
            for i in range(4):
                nc.tensor.transpose(_t(ps[:, i * P:(i + 1) * P]),
                                    src[:, i, j * P:(j + 1) * P],
                                    ident[:])
            evac(j, ps)

    def transpose512b(src, dst):
        """dst = src^T for bf16 tiles: PE transposes with a bf16 view of
        the f32 psum bank (1 cycle/row + cheap bf16 identity reload)."""
        for j in range(4):
            ps = pstt.tile([P, 512], F32, tag="pstt")
            psb = ps.bitcast(BF16)
            for i in range(4):
                nc.tensor.transpose(psb[:, i * P:(i + 1) * P],
                                    src[:, i, j * P:(j + 1) * P],
                                    identb[:])
            nc.scalar.copy(dst[:, j, :], psb[:, 0:512])

    def dma_transpose512(src, dst):
        """dst = src^T (512x512, 2-byte dtype) via 4 XBAR DMA transposes."""
        for k in range(4):
            nc.sync.dma_start_transpose(
                dst[:, :, k * P:(k + 1) * P], src[:, k, :])

    xcs = {}

    def load_xc(n):
        t = xpool.tile([P, KC, H], MDT, tag="xc", name="xc")
        for k in range(KC):
            nc.sync.dma_start(t[:, k, :],
                              x[n][k * P:(k + 1) * P, :].bitcast(MDT))
        xcs[n] = t

    heads = {}

    def do_head(n):
        """Emit item n's preamble: t1 chain + x transpose (PE-heavy).

        Called early for item 0, then from inside item n-1's MM7->MM8
        latency gap so the PE never idles at item boundaries.
        """
        xc = xcs[n]
        # t1 = p1 . x  -> [1, H] psum
        ps_t1 = psmisc.tile([1, H], F32, tag="psmisc", name="ps_t1")
        for k in range(KC):
            nc.tensor.matmul(ps_t1[:], p1_t[:, k:k + 1], xc[:, k, :],
                             start=(k == 0), stop=(k == KC - 1))
        t1row = small.tile([1, H], MDT, tag="t1row", name="t1row")
        nc.scalar.activation(t1row[:], ps_t1[:], AF.Identity)

        # t1 as per-partition column [P, KH] (f32 transpose: f32r is
        # invalid ISA for tiny single-column transposes)
        ps_t1h = psmisc.tile([P, KH], F32, tag="psmisc", name="ps_t1h")
        for j in range(KH):
            nc.tensor.transpose(ps_t1h[:, j:j + 1],
                                _f(t1row[0:1, j * P:(j + 1) * P]),
                                identf[0:1, 0:1])
        t1h = small.tile([P, KH], F32, tag="t1h", name="t1h")
        nc.vector.tensor_copy(t1h[:], ps_t1h[:])

        # x transposed on PE: xh[h, c]; t5h = relu from the same psum
        xh = big.tile([P, KH, C], MDT, tag="xh", name="xh")
        t5h = big.tile([P, KH, C], MDT, tag="t5h", name="t5h")
        for j in range(KH):
            ps = pstt.tile([P, C], F32, tag="pstt", name="ps_xh")
            for i in range(KC):
                nc.tensor.transpose(_t(ps[:, i * P:(i + 1) * P]),
                                    xc[:, i, j * P:(j + 1) * P],
                                    ident[:])
            nc.vector.tensor_copy(xh[:, j, :], ps[:])
            nc.vector.tensor_scalar_max(t5h[:, j, :], ps[:], 0.0)
        heads[n] = (t1h, xh, t5h)

    mm1s = {}

    def do_mm1(n):
        """Emit t5c/t7c (Pool) + MM1 for item n; called from inside item
        n-1's MM7->MM8 latency gap to keep the PE fed."""
        xc = xcs[n]
        t5c = big.tile([P, KC, H], MDT, tag="t5c", name="t5c")
        t7c = big.tile([P, KC, H], BF16, tag="t7c", name="t7c")
        for k in range(KC):
            nc.scalar.activation(t5c[:, k, :], _f(xc[:, k, :]), AF.Relu)
            eng_mul = nc.vector if NO_POOL else nc.gpsimd
            eng_mul.tensor_mul(t7c[:, k, :], p7c[:, k, :],
                               _f(t5c[:, k, :]))
        esum8 = small.tile([P, KH], F32, tag="esum8", name="esum8")
        # MM1: t8_un = exp(t5^T x / sqrt(c))           [h, g]
        t8 = big.tile([P, KH, H], MMDT, tag="t8", name="t8")
        mm512(t5c, xc, expevac(t8, INV_SC, esum8))
        rr8 = small.tile([P, KH], F32, tag="rr8", name="rr8")
        nc.vector.reciprocal(rr8[:], esum8[:])
        mm1s[n] = (t7c, t8, rr8)

    load_xc(0)
    nc.sync.dma_start(p7c[:], p7.rearrange("(k p) h -> p k h", p=P))
    do_head(0)
    do_mm1(0)
    # p7 transposed once per core (feeds Pool muls; plain f32).
    # Emitted after item 0's head so the PE starts on t1/xh first.
    for j in range(KH):
        ps = pstt.tile([P, C], F32, tag="pstt")
        for i in range(KC):
            nc.tensor.transpose(ps[:, i * P:(i + 1) * P],
                                p7c[:, i, j * P:(j + 1) * P], identf[:])
        nc.scalar.activation(p7h[:, j, :], ps[:], AF.Identity)
    for n in range(NB):
        xc = xcs.pop(n)
        t1h, xh, t5h = heads.pop(n)
        t7c, t8, rr8 = mm1s.pop(n)

        # prefetch next item's x while this item computes
        if n + 1 < NB:
            load_xc(n + 1)

        # H-layout elementwise (f32r: bf16 reads are slow on DVE)
        t3h = big.tile([P, KH, C], MDT, tag="t3h")
        t4h = big.tile([P, KH, C], MDT, tag="t4h")
        t7h = big.tile([P, KH, C], MDT, tag="t7h")
        t11h = big.tile([P, KH, C], MDT, tag="t11h")
        for j in range(KH):
            # t3h = roll(x,2,axis=c) + t1  (roll => free-dim shift in H)
            nc.vector.tensor_scalar_add(t3h[:, j, SHIFT:C],
                                        _f(xh[:, j, 0:C - SHIFT]),
                                        t1h[:, j:j + 1])
            nc.vector.tensor_scalar_add(t3h[:, j, 0:SHIFT],
                                        _f(xh[:, j, C - SHIFT:C]),
                                        t1h[:, j:j + 1])
            nc.vector.tensor_mul(t4h[:, j, :], _f(xh[:, j, :]),
                                 _f(t3h[:, j, :]))
            eng_mul = nc.vector if NO_POOL else nc.gpsimd
            eng_mul.tensor_mul(t7h[:, j, :], p7h[:, j, :],
                               _f(t5h[:, j, :]))
            nc.vector.tensor_tensor(t11h[:, j, :], _f(t4h[:, j, :]),
                                    _f(xh[:, j, :]), op=ALU.min)

        # t2 = roll(x, 2, axis=c): loaded straight from DRAM with row
        # shift (NO SBUF->SBUF DMA: those deadlock against XBAR
        # DMA-transposes on HW)
        t2c = big.tile([P, KC, H], MDT, tag="t2c")
        nc.sync.dma_start(
            t2c[SHIFT:P, 0, :],
            x[n][0:P - SHIFT, :].bitcast(MDT))
        nc.sync.dma_start(
            t2c[:, 1:KC, :],
            x[n][P - SHIFT:C - SHIFT, :].rearrange(
                "(k p) h -> p k h", p=P).bitcast(MDT))
        nc.sync.dma_start(
            t2c[0:SHIFT, 0, :],
            x[n][C - SHIFT:C, :].bitcast(MDT))

        # t11c = t11h^T on PE
        t11c = big.tile([P, KC, H], MDT, tag="t11c")
        transpose512(t11h, t11c, evac_dve(t11c, 1.0))

        # softmax row-sum tiles
        esum10 = small.tile([P, KC], F32, tag="esum10")
        esum15 = small.tile([P, KH], F32, tag="esum15")
        esum14 = small.tile([P, KC], F32, tag="esum14")

        # MM2: t9_raw = t5^T_h t4_h  (true t9 = raw * inv_sh)  [c, d]
        t9 = big.tile([P, KC, C], BF16, tag="t9")
        mm512(t5h, t4h, evac_dve(t9, 1.0))

        # MM3: t10_un = exp(t7 t3^T / sqrt(h)); norm on Pool    [c, d]
        t10 = big.tile([P, KC, C], BF16, tag="t10")
        mm512(t7h, t3h, expevac(t10, INV_SH, esum10))
        rr10 = small.tile([P, KC], F32, tag="rr10")
        nc.vector.reciprocal(rr10[:], esum10[:])
        for m in range(KC):
            nc.scalar.mul(t10[:, m, :], t10[:, m, :], rr10[:, m:m + 1])

        # MM5: t13T_un = t2^T t11 * r8-rows              [g, h]
        t13T = big.tile([P, KH, H], MMDT, tag="t13T")
        mm512(t2c, t11c, evac_act_ap(t13T, rr8))

        # t10T = t10^T (normalized)
        t10T = big.tile([P, KC, C], BF16, tag="t10T")
        transpose512b(t10, t10T)

        # MM4: t12 = t9^T t7 / (sqrt(c) sqrt(h))         [i, h]
        t12 = big.tile([P, KC, H], BF16, tag="t12")
        mm512(t9, t7c, evac_dve(t12, INV_SC * INV_SH))

        # MM6: t15_un = exp(t13 t8 / (sqrt(c) sqrt(h)))  [i, k]
        t15 = big.tile([P, KH, H], MDT, tag="t15")
        mm512(t13T, t8, expevac(t15, INV_SC * INV_SH, esum15))
        # rr15c = 1/(esum15*sqrt(c)) folds MM8's inv_sc with r15
        esum15s = small.tile([P, KH], F32, tag="esum15s")
        nc.vector.tensor_scalar_mul(esum15s[:], esum15[:], SC)
        rr15c = small.tile([P, KH], F32, tag="rr15c")
        nc.vector.reciprocal(rr15c[:], esum15s[:])

        # MM7: t14_un = exp(t9^T t10^T / (sqrt(c) sqrt(h))) [i, j]
        t14 = big.tile([P, KC, C], BF16, tag="t14")
        mm512(t9, t10T, expevac(t14, INV_SC * INV_SH, esum14))
        esum14s = small.tile([P, KC], F32, tag="esum14s")
        nc.vector.tensor_scalar_mul(esum14s[:], esum14[:], SH)
        rr14c = small.tile([P, KC], F32, tag="rr14c")
        nc.vector.reciprocal(rr14c[:], esum14s[:])

        # fill the MM7->t14T latency gap with the next item's preamble
        # and its MM1 (PE work that hides the t14T DMA latency)
        if n + 1 < NB:
            do_head(n + 1)
            do_mm1(n + 1)

        # t14T
        t14T = big.tile([P, KC, C], BF16, tag="t14T")
        transpose512b(t14, t14T)

        # MM8: t16 = t12^T t14T * (inv_sc * r15-rows)    [h, j]
        t16 = big.tile([P, KH, C], MDT, tag="t16")
        mm512(t12, t14T, evac_act_ap(t16, rr15c))

        # MM9: t17 = t16^T t15 * (inv_sh * r14-rows); per-chunk DMA out
        t17 = big.tile([P, KC, H], F32, tag="t17")

        def evac_out(m, ps):
            nc.scalar.activation(t17[:, m, :], ps[:], AF.Copy,
                                 scale=rr14c[:, m:m + 1])
            nc.sync.dma_start(out[n][m * P:(m + 1) * P, :], t17[:, m, :])

        mm512(t16, t15, evac_out)

    ctx.close()


_NC_CACHE = {}


def _get_program():
    if "nc" not in _NC_CACHE:
        _NC_CACHE["nc"] = build_program()
    return _NC_CACHE["nc"]


def kernel(x, p1_w, p7_w):
    x = np.ascontiguousarray(x, dtype=np.float32)
    p1 = np.ascontiguousarray(p1_w, dtype=np.float32)
    p7 = np.ascontiguousarray(np.asarray(p7_w).reshape(C, H),
                              dtype=np.float32)
    nc = _get_program()
    in_maps = [
        {"x": np.ascontiguousarray(x[i * NB:(i + 1) * NB]),
         "p1": p1, "p7": p7}
        for i in range(NCORES)
    ]
    res = bass_utils.run_bass_kernel_spmd(nc, in_maps,
                                          core_ids=list(range(NCORES)))
    outs = [np.asarray(res.results[i]["out"]) for i in range(NCORES)]
    return np.concatenate(outs, axis=0)


if __name__ == "__main__":
    nc = build_program()
    print("built ok")


# revision 51
# speedup vs baseline: 1.3525x; 1.0027x over previous
"""Trainium2 Bass kernel for nn_Kernel_503460026817608471_12541304504791.

Full-input contract: kernel(x=(64,512,512) f32, p1_w=(512,1) f32,
p7_w=(1,512,512) f32) -> (64,512,512) f32.

Pure data parallel over batch (8 items per NeuronCore). Per item
(C = H = 512, P = 128, 4 partition chunks per matrix):

  t1  = p1.x          (PE matvec)
  t2  = roll(x,2,c)   (C-layout: 3 partition-shifting SBUF DMAs)
  t3  = t1 + t2; t4 = x*t3; t5 = relu(x); t7 = p7*t5; t11 = min(t4,x)
  9 512^3 matmuls (f32r / bf16 operands, 1 PE cycle/row), 4 softmaxes.

Softmaxes skip max-subtraction (logits are bounded, max ~26) and their
row normalizers are folded into downstream per-partition evac scales:
  r8  -> t13T evac   (t8 used as MM6 rhs; t13T rows share the index)
  r15 -> t16 evac    (t15 used as MM9 rhs)
  r14 -> t17 evac    (t14T free dim becomes MM9 psum partition)
  r10 -> explicit in-place mul on GpSimd (transpose blocks folding)

Engine balance: PE ~35us/item (bottleneck), ACT exps+evacs, DVE
elementwise+evacs, GpSimd (Pool) p7 muls + t10 norm, Sync 6 DMAs.
"""

import math
import os
import sys

for _p in ("/opt/trn_rl_repo",):
    if _p not in sys.path and os.path.isdir(_p):
        sys.path.insert(0, _p)

import numpy as np

import concourse.bass as bass
import concourse.tile as tile
from concourse import bacc
from concourse import mybir
from concourse import bass_utils
from concourse.masks import make_identity

N, C, H = 64, 512, 512
SHIFT = 2
NCORES = 8
NB = N // NCORES          # batch items per core
P = 128
KC = C // P               # 4 chunks along c
KH = H // P               # 4 chunks along h

F32 = mybir.dt.float32
F32R = mybir.dt.float32r
BF16 = mybir.dt.bfloat16
AF = mybir.ActivationFunctionType
ALU = mybir.AluOpType
AX = mybir.AxisListType

MDT = F32R
SC = math.sqrt(C)
SH = math.sqrt(H)
INV_SC = 1.0 / SC
INV_SH = 1.0 / SH

# Config knobs. Defaults chosen from HW bisection: XBAR DMA-transposes
# corrupt data when concurrent with other DMA traffic in this kernel
# (known TRN2 erratum: Tile normally serializes them), and without
# them bf16 matmul operands buy nothing (PE is 1 cycle/row for f32r
# and bf16 alike, while DVE *reads* of bf16 are ~4x slower).
NO_DMAT = os.environ.get("K_NO_DMAT", "1") == "1"   # PE transposes
NO_POOL = os.environ.get("K_NO_POOL", "0") == "1"   # DVE muls instead
NO_BF16 = os.environ.get("K_NO_BF16", "1") == "1"   # f32r matmuls only
MMDT = F32R if NO_BF16 else BF16


def _f(ap):
    """f32 read view of an f32r tile for elementwise ops."""
    return ap.bitcast(F32)


def _t(ap):
    """Transpose PSUM output view (must match transpose input dtype)."""
    return ap.bitcast(MDT)


def build_program():
    nc = bacc.Bacc("TRN2", target_bir_lowering=False, debug=False,
                   num_devices=NCORES)
    x = nc.dram_tensor("x", [NB, C, H], F32, kind="ExternalInput").ap()
    p1 = nc.dram_tensor("p1", [C, 1], F32, kind="ExternalInput").ap()
    p7 = nc.dram_tensor("p7", [C, H], F32, kind="ExternalInput").ap()
    out = nc.dram_tensor("out", [NB, C, H], F32, kind="ExternalOutput").ap()

    with tile.TileContext(nc) as tc:
        _build_body(nc, tc, x, p1, p7, out)
    nc.compile()
    return nc


def _build_body(nc, tc, x, p1, p7, out):
    from contextlib import ExitStack
    ctx = ExitStack()
    constp = ctx.enter_context(tc.tile_pool(name="const", bufs=1))
    big = ctx.enter_context(tc.tile_pool(name="big", bufs=1))
    xpool = ctx.enter_context(tc.tile_pool(name="xc", bufs=2))
    small = ctx.enter_context(tc.tile_pool(name="small", bufs=2))
    psmm = ctx.enter_context(
        tc.tile_pool(name="psmm", bufs=5, space="PSUM"))
    pstt = ctx.enter_context(
        tc.tile_pool(name="pstt", bufs=2, space="PSUM"))
    psmisc = ctx.enter_context(
        tc.tile_pool(name="psmisc", bufs=1, space="PSUM"))

    identf = constp.tile([P, P], F32)
    make_identity(nc, identf[:])
    ident = constp.tile([P, P], MDT)
    nc.scalar.activation(ident[:], identf[:], AF.Identity)
    identb = constp.tile([P, P], BF16)
    nc.scalar.activation(identb[:], identf[:], AF.Identity)

    p1_f = constp.tile([P, KC], F32)
    nc.sync.dma_start(p1_f[:], p1.rearrange("(k p) o -> p (k o)", p=P))
    p1_t = constp.tile([P, KC], MDT)
    nc.scalar.activation(p1_t[:], p1_f[:], AF.Identity)
    p7c = constp.tile([P, KC, H], F32)
    p7h = constp.tile([P, KH, C], F32)

    def mm512(lhsT, rhs, consume):
        """out[m,n] = sum_k lhsT[k,m] * rhs[k,n], 512^3, via 4x4 PE matmuls.
        lhsT/rhs are [P, 4, 512] chunk-major tiles; consume(m, psum)."""
        for m in range(4):
            ps = psmm.tile([P, 512], F32, tag="psmm")
            for k in range(4):
                nc.tensor.matmul(ps[:],
                                 lhsT[:, k, m * P:(m + 1) * P],
                                 rhs[:, k, :],
                                 start=(k == 0), stop=(k == 3))
            consume(m, ps)

    def evac_act(dst, scale):
        def f(m, ps):
            nc.scalar.activation(dst[:, m, :], ps[:], AF.Identity,
                                 scale=scale)
        return f

    def evac_act_ap(dst, rr):
        """ACT evac scaled by per-partition column rr[:, m]."""
        def f(m, ps):
            nc.scalar.activation(dst[:, m, :], ps[:], AF.Copy,
                                 scale=rr[:, m:m + 1])
        return f

    def evac_dve(dst, scale):
        def f(m, ps):
            if scale == 1.0:
                nc.vector.tensor_copy(dst[:, m, :], ps[:])
            else:
                nc.vector.tensor_scalar_mul(dst[:, m, :], ps[:], scale)
        return f

    def evac_dve_ap(dst, rr):
        """DVE evac scaled by per-partition column rr[:, m]."""
        def f(m, ps):
            nc.vector.tensor_scalar_mul(dst[:, m, :], ps[:],
                                        rr[:, m:m + 1])
        return f

    def expevac(dst, scale, esum):
        """dst[:,m,:] = exp(scale*psum); row sums accumulated in esum."""
        def f(m, ps):
            nc.scalar.activation(dst[:, m, :], ps[:], AF.Exp,
                                 scale=scale, accum_out=esum[:, m:m + 1])
        return f

    def transpose512(src, dst, evac):
        """dst = src^T (512x512) via 16 PE transposes; evac(j, psum)."""
        for j in range(4):
            ps = pstt.tile([P, 512], F32, tag="pstt")
            for i in range(4):
                nc.tensor.transpose(_t(ps[:, i * P:(i + 1) * P]),
                                    src[:, i, j * P:(j + 1) * P],
                                    ident[:])
            evac(j, ps)

    def transpose512b(src, dst):
        """dst = src^T for bf16 tiles: PE transposes with a bf16 view of
        the f32 psum bank (1 cycle/row + cheap bf16 identity reload)."""
        for j in range(4):
            ps = pstt.tile([P, 512], F32, tag="pstt")
            psb = ps.bitcast(BF16)
            for i in range(4):
                nc.tensor.transpose(psb[:, i * P:(i + 1) * P],
                                    src[:, i, j * P:(j + 1) * P],
                                    identb[:])
            nc.scalar.copy(dst[:, j, :], psb[:, 0:512])

    def dma_transpose512(src, dst):
        """dst = src^T (512x512, 2-byte dtype) via 4 XBAR DMA transposes."""
        for k in range(4):
            nc.sync.dma_start_transpose(
                dst[:, :, k * P:(k + 1) * P], src[:, k, :])

    xcs = {}

    def load_xc(n):
        t = xpool.tile([P, KC, H], MDT, tag="xc", name="xc")
        for k in range(KC):
            nc.sync.dma_start(t[:, k, :],
                              x[n][k * P:(k + 1) * P, :].bitcast(MDT))
        xcs[n] = t

    heads = {}

    def do_head(n):
        """Emit item n's preamble: t1 chain + x transpose (PE-heavy).

        Called early for item 0, then from inside item n-1's MM7->MM8
        latency gap so the PE never idles at item boundaries.
        """
        xc = xcs[n]
        # t1 = p1 . x  -> [1, H] psum
        ps_t1 = psmisc.tile([1, H], F32, tag="psmisc", name="ps_t1")
        for k in range(KC):
            nc.tensor.matmul(ps_t1[:], p1_t[:, k:k + 1], xc[:, k, :],
                             start=(k == 0), stop=(k == KC - 1))
        t1row = small.tile([1, H], MDT, tag="t1row", name="t1row")
        nc.scalar.activation(t1row[:], ps_t1[:], AF.Identity)

        # t1 as per-partition column [P, KH] (f32 transpose: f32r is
        # invalid ISA for tiny single-column transposes)
        ps_t1h = psmisc.tile([P, KH], F32, tag="psmisc", name="ps_t1h")
        for j in range(KH):
            nc.tensor.transpose(ps_t1h[:, j:j + 1],
                                _f(t1row[0:1, j * P:(j + 1) * P]),
                                identf[0:1, 0:1])
        t1h = small.tile([P, KH], F32, tag="t1h", name="t1h")
        nc.vector.tensor_copy(t1h[:], ps_t1h[:])

        # bf16 copy of x: feeds the t2c roll and (via t11h) MM5
        xb = big.tile([P, KC, H], BF16, tag="xb", name="xb")
        for k in range(KC):
            nc.vector.tensor_copy(xb[:, k, :], _f(xc[:, k, :]))

        # x transposed on PE: xh[h, c]; t5h = relu from the same psum
        xh = big.tile([P, KH, C], MDT, tag="xh", name="xh")
        t5h = big.tile([P, KH, C], MDT, tag="t5h", name="t5h")
        for j in range(KH):
            ps = pstt.tile([P, C], F32, tag="pstt", name="ps_xh")
            for i in range(KC):
                nc.tensor.transpose(_t(ps[:, i * P:(i + 1) * P]),
                                    xc[:, i, j * P:(j + 1) * P],
                                    ident[:])
            nc.vector.tensor_copy(xh[:, j, :], ps[:])
            nc.vector.tensor_scalar_max(t5h[:, j, :], ps[:], 0.0)
        heads[n] = (t1h, xh, t5h, xb)

    mm1s = {}

    def do_mm1(n):
        """Emit t5c/t7c (Pool) + MM1 for item n; called from inside item
        n-1's MM7->MM8 latency gap to keep the PE fed."""
        xc = xcs[n]
        t5c = big.tile([P, KC, H], MDT, tag="t5c", name="t5c")
        t7c = big.tile([P, KC, H], BF16, tag="t7c", name="t7c")
        for k in range(KC):
            nc.vector.tensor_scalar_max(t5c[:, k, :], _f(xc[:, k, :]),
                                        0.0)
            eng_mul = nc.vector if NO_POOL else nc.gpsimd
            eng_mul.tensor_mul(t7c[:, k, :], p7c[:, k, :],
                               _f(t5c[:, k, :]))
        esum8 = small.tile([P, KH], F32, tag="esum8", name="esum8")
        # MM1: t8_un = exp(t5^T x / sqrt(c))           [h, g]
        t8 = big.tile([P, KH, H], MMDT, tag="t8", name="t8")
        mm512(t5c, xc, expevac(t8, INV_SC, esum8))
        rr8 = small.tile([P, KH], F32, tag="rr8", name="rr8")
        nc.vector.reciprocal(rr8[:], esum8[:])
        mm1s[n] = (t7c, t8, rr8)

    load_xc(0)
    nc.sync.dma_start(p7c[:], p7.rearrange("(k p) h -> p k h", p=P))
    do_head(0)
    do_mm1(0)
    # p7 transposed once per core (feeds Pool muls; plain f32).
    # Emitted after item 0's head so the PE starts on t1/xh first.
    for j in range(KH):
        ps = pstt.tile([P, C], F32, tag="pstt")
        for i in range(KC):
            nc.tensor.transpose(ps[:, i * P:(i + 1) * P],
                                p7c[:, i, j * P:(j + 1) * P], identf[:])
        nc.scalar.activation(p7h[:, j, :], ps[:], AF.Identity)
    for n in range(NB):
        xc = xcs.pop(n)
        t1h, xh, t5h, xb = heads.pop(n)
        t7c, t8, rr8 = mm1s.pop(n)

        # prefetch next item's x while this item computes
        if n + 1 < NB:
            load_xc(n + 1)

        # H-layout elementwise (f32r: bf16 reads are slow on DVE)
        t3h = big.tile([P, KH, C], MDT, tag="t3h")
        t4h = big.tile([P, KH, C], MDT, tag="t4h")
        t7h = big.tile([P, KH, C], MDT, tag="t7h")
        t11h = big.tile([P, KH, C], BF16, tag="t11h")
        for j in range(KH):
            # t3h = roll(x,2,axis=c) + t1  (roll => free-dim shift in H)
            nc.vector.tensor_scalar_add(t3h[:, j, SHIFT:C],
                                        _f(xh[:, j, 0:C - SHIFT]),
                                        t1h[:, j:j + 1])
            nc.vector.tensor_scalar_add(t3h[:, j, 0:SHIFT],
                                        _f(xh[:, j, C - SHIFT:C]),
                                        t1h[:, j:j + 1])
            nc.vector.tensor_mul(t4h[:, j, :], _f(xh[:, j, :]),
                                 _f(t3h[:, j, :]))
            eng_mul = nc.vector if NO_POOL else nc.gpsimd
            eng_mul.tensor_mul(t7h[:, j, :], p7h[:, j, :],
                               _f(t5h[:, j, :]))
            nc.vector.tensor_tensor(t11h[:, j, :], _f(t4h[:, j, :]),
                                    _f(xh[:, j, :]), op=ALU.min)

        # t2 = roll(x, 2, axis=c): loaded straight from DRAM with row
        # shift (NO SBUF->SBUF DMA: those deadlock against XBAR
        # DMA-transposes on HW)
        t2c = big.tile([P, KC, H], BF16, tag="t2c")
        nc.sync.dma_start(t2c[SHIFT:P, :, :], xb[0:P - SHIFT, :, :])
        nc.sync.dma_start(t2c[0:SHIFT, 1:KC, :],
                          xb[P - SHIFT:P, 0:KC - 1, :])
        nc.sync.dma_start(t2c[0:SHIFT, 0, :], xb[P - SHIFT:P, KC - 1, :])

        # t11c = t11h^T on PE (bf16)
        t11c = big.tile([P, KC, H], BF16, tag="t11c")
        transpose512b(t11h, t11c)

        # softmax row-sum tiles
        esum10 = small.tile([P, KC], F32, tag="esum10")
        esum15 = small.tile([P, KH], F32, tag="esum15")
        esum14 = small.tile([P, KC], F32, tag="esum14")

        # MM2: t9_raw = t5^T_h t4_h  (true t9 = raw * inv_sh)  [c, d]
        t9 = big.tile([P, KC, C], BF16, tag="t9")
        mm512(t5h, t4h, evac_dve(t9, 1.0))

        # MM3: t10_un = exp(t7 t3^T / sqrt(h)); norm on Pool    [c, d]
        t10 = big.tile([P, KC, C], BF16, tag="t10")
        mm512(t7h, t3h, expevac(t10, INV_SH, esum10))
        rr10 = small.tile([P, KC], F32, tag="rr10")
        nc.vector.reciprocal(rr10[:], esum10[:])
        for m in range(KC):
            nc.scalar.mul(t10[:, m, :], t10[:, m, :], rr10[:, m:m + 1])

        # MM5: t13T_un = t2^T t11 * r8-rows              [g, h]
        t13T = big.tile([P, KH, H], MMDT, tag="t13T")
        mm512(t2c, t11c, evac_dve_ap(t13T, rr8))

        # t10T = t10^T (normalized)
        t10T = big.tile([P, KC, C], BF16, tag="t10T")
        transpose512b(t10, t10T)

        # MM4: t12 = t9^T t7 / (sqrt(c) sqrt(h))         [i, h]
        t12 = big.tile([P, KC, H], BF16, tag="t12")
        mm512(t9, t7c, evac_dve(t12, INV_SC * INV_SH))

        # MM6: t15_un = exp(t13 t8 / (sqrt(c) sqrt(h)))  [i, k]
        t15 = big.tile([P, KH, H], MDT, tag="t15")
        mm512(t13T, t8, expevac(t15, INV_SC * INV_SH, esum15))
        # rr15c = 1/(esum15*sqrt(c)) folds MM8's inv_sc with r15
        esum15s = small.tile([P, KH], F32, tag="esum15s")
        nc.vector.tensor_scalar_mul(esum15s[:], esum15[:], SC)
        rr15c = small.tile([P, KH], F32, tag="rr15c")
        nc.vector.reciprocal(rr15c[:], esum15s[:])

        # MM7: t14_un = exp(t9^T t10^T / (sqrt(c) sqrt(h))) [i, j]
        t14 = big.tile([P, KC, C], BF16, tag="t14")
        mm512(t9, t10T, expevac(t14, INV_SC * INV_SH, esum14))
        esum14s = small.tile([P, KC], F32, tag="esum14s")
        nc.vector.tensor_scalar_mul(esum14s[:], esum14[:], SH)
        rr14c = small.tile([P, KC], F32, tag="rr14c")
        nc.vector.reciprocal(rr14c[:], esum14s[:])

        # fill the MM7->t14T latency gap with the next item's preamble
        # and its MM1 (PE work that hides the t14T DMA latency)
        if n + 1 < NB:
            do_head(n + 1)
            do_mm1(n + 1)

        # t14T
        t14T = big.tile([P, KC, C], BF16, tag="t14T")
        transpose512b(t14, t14T)

        # MM8: t16 = t12^T t14T * (inv_sc * r15-rows)    [h, j]
        t16 = big.tile([P, KH, C], MDT, tag="t16")
        mm512(t12, t14T, evac_act_ap(t16, rr15c))

        # MM9: t17 = t16^T t15 * (inv_sh * r14-rows); per-chunk DMA out
        t17 = big.tile([P, KC, H], F32, tag="t17")

        def evac_out(m, ps):
            nc.scalar.activation(t17[:, m, :], ps[:], AF.Copy,
                                 scale=rr14c[:, m:m + 1])
            nc.sync.dma_start(out[n][m * P:(m + 1) * P, :], t17[:, m, :])

        mm512(t16, t15, evac_out)

    ctx.close()


_NC_CACHE = {}


def _get_program():
    if "nc" not in _NC_CACHE:
        _NC_CACHE["nc"] = build_program()
    return _NC_CACHE["nc"]


def kernel(x, p1_w, p7_w):
    x = np.ascontiguousarray(x, dtype=np.float32)
    p1 = np.ascontiguousarray(p1_w, dtype=np.float32)
    p7 = np.ascontiguousarray(np.asarray(p7_w).reshape(C, H),
                              dtype=np.float32)
    nc = _get_program()
    in_maps = [
        {"x": np.ascontiguousarray(x[i * NB:(i + 1) * NB]),
         "p1": p1, "p7": p7}
        for i in range(NCORES)
    ]
    res = bass_utils.run_bass_kernel_spmd(nc, in_maps,
                                          core_ids=list(range(NCORES)))
    outs = [np.asarray(res.results[i]["out"]) for i in range(NCORES)]
    return np.concatenate(outs, axis=0)


if __name__ == "__main__":
    nc = build_program()
    print("built ok")
